# revision 31
# baseline (speedup 1.0000x reference)
"""Trainium2 Bass kernel for the pickup/delivery heterogeneous MHA module.

Shapes (hardcoded): q (16, 501, 128) f32, 8 heads, key dim 16,
n_pick = n_delivery = 250, G = 1 + 250 + 250 = 501.

Sharding: data parallel over batch — 2 batches per core on 8 cores.

v4 architecture:
  - bf16 hi/lo projections and bf16 score matmuls (fp32r moving operands
    are ISA-capped at 256 columns, so the single-pass f32r idea is out).
  - combined attention weights: w[key, query] = exp(s_main) + exp(s_blk)
    is formed in SBUF (bf16 add on Pool/DVE) so attn@V needs ONE matmul
    per (head, key-chunk) instead of two — halves that PE stage.
  - exp runs on two engines: ACT (native spline exp) and DVE (Schraudolph
    bit-trick: bf16 bits = int16(x * 128/ln2 + 16256 - c), one
    tensor_scalar op). Assignment is per-head so each (head, query)
    softmax row sees one consistent exp approximation (errors cancel in
    the num/den ratio).
  - softmax denominators ride along as 16 "ones" columns in the V tiles;
    a bf16 selector matmul broadcasts them per 32-row group.
"""

import sys

for _p in ("/opt/trn_rl_repo", "/root/.axon_site/_ro/trn_rl_repo"):
    if _p not in sys.path:
        sys.path.insert(0, _p)

import math

import ml_dtypes
import numpy as np

B, G, D, H, KD = 16, 501, 128, 8, 16
NP = ND = 250
NCORES = 8
BPC = B // NCORES  # batches per core

# rotated g order: [picks (g 1..251), delivs (g 251..501), depot (g 0)]
ROT = np.concatenate([np.arange(1, G), [0]])

# key chunks (partition tiles of the key axis, rotated coords):
# (main_c0, main_c1, blk_c1, qsrc) — block keys stop at 500 (no depot);
# chunks 0/1 are picks (block queries from qp), 2/3 delivs (qd)
CHUNKS = [(0, 128, 128, "qp"), (128, 250, 250, "qp"),
          (250, 378, 378, "qd"), (378, 501, 500, "qd")]

# Schraudolph exp constants (bf16 bit trick), c tuned for min max-rel-err
SCH_A = 128.0 / math.log(2.0)
SCH_B = 127.0 * 128.0 - 5.5 + 0.5  # +0.5: trunc-to-int acts as round

# exp engine per head: "act" = native exp on ACT, "dve" = Schraudolph.
# Per-head (not per-chunk) so each softmax row is internally consistent.
# HW probe: ACT exp ~780ns/unit, DVE Schraudolph ~1057ns/unit, and Pool
# (gpsimd) tensor_tensor is ~4x the cost model — so ACT carries most exp
# and the combine-adds live on DVE, not Pool.
EXP_ENGINE = ["dve", "act", "act", "act", "act", "act", "act", "act"]
# engine for the e_comb += e_blk add, per head ("pool" or "dve")
ADD_ENGINE = ["dve"] * 8

_CACHE = {}


def _build_nc(loop_k=0, exp_engine=None, add_engine=None):
    """loop_k=0: normal kernel. loop_k>0: wrap the body in a device-side
    For_i loop of loop_k iterations (benchmarking only)."""
    import contextlib

    import concourse.bacc as bacc
    import concourse.mybir as mybir
    import concourse.tile as tile

    exp_engine = exp_engine or EXP_ENGINE
    add_engine = add_engine or ADD_ENGINE

    f32 = mybir.dt.float32
    f32r = mybir.dt.float32r
    bf16 = mybir.dt.bfloat16
    i16 = mybir.dt.int16
    EXP = mybir.ActivationFunctionType.Exp
    MULT = mybir.AluOpType.mult
    ADD = mybir.AluOpType.add

    nc = bacc.Bacc("TRN2", target_bir_lowering=False, debug=False,
                   num_devices=NCORES)

    # hq carries a hi/lo bf16 split of q^T so projections run as two
    # accumulating bf16 matmuls (fp32r moving operands are capped at 256
    # columns by the ISA, so single-pass f32r projections are out)
    hq = nc.dram_tensor("hq", [BPC, 2, D, G], bf16, kind="ExternalInput")
    # bf16 weights packed into one DMA: wq wke wko w1 w2 w3 w4 wv
    wnames = ["wq", "wke", "wko", "w1", "w2", "w3", "w4", "wv"]
    walldr = nc.dram_tensor("wall", [D, len(wnames) * D], bf16,
                            kind="ExternalInput")
    wobfdr = nc.dram_tensor("wobf", [D, 2 * D], bf16, kind="ExternalInput")
    seldr = nc.dram_tensor("sel", [D, D], bf16, kind="ExternalInput")
    out = nc.dram_tensor("out", [BPC, G, D], f32, kind="ExternalOutput")

    with tile.TileContext(nc) as tc:
        with (
            tc.tile_pool(name="const", bufs=1) as constp,
            tc.tile_pool(name="perb", bufs=2) as perb,
            tc.tile_pool(name="vext", bufs=2) as vextp,
            tc.tile_pool(name="ecomb", bufs=4) as ecombp,
            tc.tile_pool(name="tmp", bufs=3) as tmpp,
            tc.tile_pool(name="outp", bufs=3) as outp,
            tc.tile_pool(name="ps", bufs=2, space="PSUM") as psp,
            tc.tile_pool(name="hps", bufs=2, space="PSUM") as hpsp,
        ):
            # hq[0] rides the SP queue FIRST: the first projection blocks
            # on it, while wall is only needed ~1.3us later
            hT0 = perb.tile([D, 2, G], bf16, name="hT")
            src0 = hq.ap()[0].transpose([1, 0, 2])
            nc.sync.dma_start(hT0[:, 0, :], src0[:, 0, :])
            nc.sync.dma_start(hT0[:, 1, :], src0[:, 1, :])
            wall = constp.tile([D, len(wnames) * D], bf16, name="wall")
            nc.sync.dma_start(wall[:], walldr.ap())
            wsb = {n: wall[:, i * D:(i + 1) * D]
                   for i, n in enumerate(wnames)}
            # sel/wobf are needed late; keep them off the critical DMA queue
            sel_sb = constp.tile([D, D], bf16, name="sel_sb")
            nc.gpsimd.dma_start(sel_sb[:], seldr.ap())
            wobf = constp.tile([D, 2 * D], bf16, name="wobf")
            nc.gpsimd.dma_start(wobf[:], wobfdr.ap())
            wo0_sb = wobf[:, 0:D]
            wo1_sb = wobf[:, D:2 * D]

            # warm the ACT exp table so its one-time load overlaps the
            # initial weight DMAs instead of stalling the first real exp
            wtile = constp.tile([1, 4], f32, name="wtile")
            nc.gpsimd.memset(wtile[:], 0.0)
            wtile2 = constp.tile([1, 4], f32, name="wtile2")
            nc.scalar.activation(wtile2[:], wtile[:], EXP)
            # dummy matmuls keep PE busy during the input DMA wait so the
            # HAM clock-gate is released (2.4 GHz) before the projections
            wz = constp.tile([D, 64], bf16, name="wz")
            nc.gpsimd.memset(wz[:], 0.0)

            def pmm(dst, w, st, col0, col1, first, last=True):
                """projection matmul: accumulate W.T @ (h_hi + h_lo)."""
                hT = st["hT"]
                nc.tensor.matmul(dst, w, hT[:, 0, col0:col1],
                                 start=first, stop=False,
                                 skip_group_check=True)
                nc.tensor.matmul(dst, w, hT[:, 1, col0:col1],
                                 start=False, stop=last,
                                 skip_group_check=True)

            def proj_dma(b, st):
                if b == 0:
                    st["hT"] = hT0
                    return
                hT = perb.tile([D, 2, G], bf16, name="hT")
                srcb = hq.ap()[b].transpose([1, 0, 2])
                nc.sync.dma_start(hT[:, 0, :], srcb[:, 0, :])
                nc.sync.dma_start(hT[:, 1, :], srcb[:, 1, :])
                st["hT"] = hT

            def proj_qkk(b, st, part=2):
                """qt | kte | kto: hi/lo bf16 accumulating matmuls.
                PSUM->SBUF copies split across ACT and DVE (Pool and DMA
                have no PSUM port) to keep DVE free for exp work."""
                names = [("wq", "qt"), ("wke", "kte"), ("wko", "kto")]
                if part == 0:
                    names = names[:1]
                elif part == 1:
                    names = names[1:]
                for wn, tn in names:
                    ps = psp.tile([128, 2, 512], f32, tag="sc", bufs=3,
                                  name=f"{tn}_ps")
                    pmm(ps[:, 0, 0:G], wsb[wn], st, 0, G, True)
                    t = perb.tile([D, G], bf16, name=tn)
                    if tn in ("qt", "kte"):
                        nc.scalar.copy(t[:], ps[:, 0, 0:G])
                    else:
                        nc.vector.tensor_copy(t[:], ps[:, 0, 0:G])
                    st[tn] = t

            def proj_qpqd(b, st, which):
                """qp = W1(picks)|W3(delivs), qd = W2|W4 — block queries.
                The second matmul runs through column 501 (a projection of
                the depot query) so the fused unit exp can cover a full
                [*, 2, 501] region; that column's weights land in an
                unused slot of the block sub-tile."""
                wa, wb = ("w1", "w3") if which == "qp" else ("w2", "w4")
                ps = psp.tile([128, 2, 512], f32, tag="sc", bufs=3,
                              name=f"{which}_ps")
                pmm(ps[:, 0, 0:250], wsb[wa], st, 0, 250, True, last=False)
                pmm(ps[:, 0, 250:501], wsb[wb], st, 250, 501, False)
                t = perb.tile([D, 512], bf16, name=which)
                nc.vector.tensor_copy(t[:, 0:501], ps[:, 0, 0:501])
                st[which] = t

            def proj_v(b, st):
                """V chunks: hi/lo bf16, hT chunk stationary, wv moving."""
                hT = st["hT"]
                v_ps = psp.tile([128, 2, 512], f32, tag="sc", bufs=3,
                                name="v_ps")
                st["v_ps"] = v_ps
                for ci, (c0, c1, _, _) in enumerate(CHUNKS):
                    for i in range(2):
                        nc.tensor.matmul(
                            v_ps[0:c1 - c0, ci // 2,
                                 128 * (ci % 2):128 * (ci % 2) + 128],
                            hT[:, i, c0:c1], wsb["wv"],
                            start=(ci % 2 == 0 and i == 0),
                            stop=(ci % 2 == 1 and i == 1),
                            skip_group_check=True)

            def proj_vcopy(b, st):
                """V_ext tiles: per chunk [128, 8 heads * (16 V + 16
                ones)]; the ones columns make the attn@V matmuls
                accumulate softmax denominators for free."""
                v_ps = st["v_ps"]
                vext = []
                for ci, (c0, c1, _, _) in enumerate(CHUNKS):
                    cs = c1 - c0
                    vt = vextp.tile([128, 256], bf16, tag=f"v{ci}",
                                    name=f"vext{ci}")
                    vv = vt.rearrange("p (h w) -> p h w", h=H)
                    nc.gpsimd.memset(vv[:, :, 16:32], 1.0)
                    src = v_ps[0:cs, ci // 2,
                               128 * (ci % 2):128 * (ci % 2) + 128]
                    nc.vector.tensor_copy(
                        vv[0:cs, :, 0:16],
                        src.rearrange("p (h v) -> p h v", h=H))
                    vext.append(vt)
                st["vext"] = vext

            def stream_units(b, st):
                """32 units = (head, key-chunk). Each: 2 score matmuls
                (main + block) -> 2 exps -> combine add -> 1 attn@V."""
                H0 = hpsp.tile([128, 512], f32, tag="H", name="H0")
                H1 = hpsp.tile([128, 512], f32, tag="H", name="H1")
                st["Hs"] = (H0, H1)

                # quad-major, chunk-rotated: heads cycle within each quad
                # so the DVE-exp head's units spread evenly instead of
                # bursting, and each quad still finishes as a block (the
                # tails hook in right after units 15 / 31)
                order = []
                for quad in range(2):
                    for ci in range(4):
                        for hh in range(4):
                            order.append((4 * quad + hh, ci))

                def unit(u):
                    h, ci = order[u]
                    c0, c1, b1, qsrc = CHUNKS[ci]
                    cs, csb = c1 - c0, b1 - c0
                    p = h // 2
                    kt = st["kte"] if h % 2 == 0 else st["kto"]
                    box = {}

                    def emit_scores():
                        sc = psp.tile([128, 2, 512], f32, tag="sc",
                                      bufs=3, name="sc")
                        box["sc"] = sc
                        kts = kt[32 * p:32 * p + 32, :]
                        nc.tensor.matmul(
                            sc[0:cs, 0, 0:G], kts[:, c0:c1],
                            st["qt"][32 * p:32 * p + 32, :],
                            tile_position=(32 * p, 0))
                        # block scores: stationary padded to cs columns
                        # (chunk 3 row 122 = depot-key row: written from
                        # real K data but never consumed); moving padded
                        # to 501 via the zeroed qp/qd column so ONE fused
                        # exp can cover [0:cs, 0:2, 0:501]
                        nc.tensor.matmul(
                            sc[0:cs, 1, 0:501], kts[:, c0:c1],
                            st[qsrc][32 * p:32 * p + 32, 0:501],
                            tile_position=(32 * p, 0))

                    def emit_exp():
                        sc = box["sc"]
                        ec = ecombp.tile([128, 2, 512], bf16, name="ec")
                        box["ec"] = ec
                        if exp_engine[h] == "act":
                            nc.scalar.activation(ec[0:cs, :, 0:G],
                                                 sc[0:cs, :, 0:G], EXP)
                        else:
                            nc.vector.tensor_scalar(
                                ec.bitcast(i16)[0:cs, :, 0:G],
                                sc[0:cs, :, 0:G], SCH_A, SCH_B, MULT, ADD)

                    def emit_add():
                        # DVE-exp heads fold the block weights into sub0
                        # right on the exp queue (cheap 2x bf16 add, no
                        # extra cross-engine hop). ACT heads skip the add:
                        # their block weights ride a second accumulating
                        # attn@V matmul instead (PE absorbs it; the
                        # shorter chain pipelines better on HW).
                        if exp_engine[h] != "dve":
                            return
                        ec = box["ec"]
                        nc.vector.tensor_tensor(ec[0:csb, 0, 0:500],
                                                ec[0:csb, 0, 0:500],
                                                ec[0:csb, 1, 0:500], ADD)

                    def emit_av():
                        ec = box["ec"]
                        vt = st["vext"][ci]
                        Hq = st["Hs"][h // 4]
                        cg = 32 * (h % 4)
                        combined = exp_engine[h] == "dve"
                        nc.tensor.matmul(
                            Hq[cg:cg + 32, 0:G],
                            vt[0:cs, 32 * h:32 * h + 32], ec[0:cs, 0, 0:G],
                            start=(ci == 0), stop=(ci == 3 and combined),
                            tile_position=(0, cg),
                            skip_group_check=True)
                        if not combined:
                            nc.tensor.matmul(
                                Hq[cg:cg + 32, 0:500],
                                vt[0:csb, 32 * h:32 * h + 32],
                                ec[0:csb, 1, 0:500],
                                start=False, stop=(ci == 3),
                                tile_position=(0, cg),
                                skip_group_check=True)

                    return (emit_scores, emit_exp, emit_add, emit_av)

                return [unit(u) for u in range(32)]

            def tail_norm_q(b, st, quad):
                """normalize one quad by its softmax denominators. The
                sel-matmul broadcasts each 32-group's denominator row to
                the whole group. All-bf16 so the sel matmul runs at
                1 cyc/row and the hn multiply hits the DVE 2x mode.
                Column-split in two halves to pipeline the serial chain."""
                Hq = st["Hs"][quad]
                hsb = tmpp.tile([D, G], bf16, tag="hsb", name="hsb")
                denb = psp.tile([128, 2, 512], f32, tag="sc", bufs=3,
                                name="denb")
                rcb = tmpp.tile([D, G], f32, tag="rcb", name="rcb")
                hn = perb.tile([D, G], bf16, name=f"hn{quad}")
                st[f"hn{quad}"] = hn
                for c0, c1 in ((0, 256), (256, G)):
                    nc.vector.tensor_copy(hsb[:, c0:c1], Hq[:, c0:c1])
                    nc.tensor.matmul(denb[:, 0, c0:c1], sel_sb,
                                     hsb[:, c0:c1], start=(c0 == 0),
                                     stop=(c0 != 0), skip_group_check=True)
                    nc.vector.reciprocal_approx_fast(rcb[:, c0:c1],
                                                     denb[:, 0, c0:c1])
                    nc.vector.tensor_mul(hn[:, c0:c1], hsb[:, c0:c1],
                                         rcb[:, c0:c1])
                    yield

            def run_gen(gen):
                for _ in gen:
                    pass

            def tail_final_mms(b, st, quad, cis=(0, 1, 2, 3)):
                """out = sum_h headsT_h @ W_out_h, accumulated across the
                two quads into two aux psum banks (2 chunk regions each)."""
                hn = st[f"hn{quad}"]
                wo = wo0_sb if quad == 0 else wo1_sb
                if "ops" not in st:
                    opt = psp.tile([128, 2, 512], f32, tag="sc", bufs=3,
                                   name="ops")
                    st["ops"] = [opt[:, 0, :], opt[:, 1, :]]
                for ci in cis:
                    c0, c1 = CHUNKS[ci][0], CHUNKS[ci][1]
                    # one start=True per BANK (ci even); the odd region
                    # relies on the bank-wide pending-zero from it
                    nc.tensor.matmul(
                        st["ops"][ci // 2][0:c1 - c0,
                                           128 * (ci % 2):128 * (ci % 2) + 128],
                        hn[:, c0:c1], wo,
                        start=(quad == 0 and ci % 2 == 0),
                        stop=(quad == 1 and ci % 2 == 1),
                        skip_group_check=True)

            def tail_final_out(b, st):
                osb = outp.tile([128, 2, 512], f32, name="osb")
                # out DMAs alternate SP / Pool queues so the final drain
                # isn't serialized on one queue
                for ci, (c0, c1, _, _) in enumerate(CHUNKS):
                    cs = c1 - c0
                    sl = slice(128 * (ci % 2), 128 * (ci % 2) + 128)
                    nc.vector.tensor_copy(osb[0:cs, ci // 2, sl],
                                          st["ops"][ci // 2][0:cs, sl])
                    reg = osb[:, ci // 2, sl]
                    q = nc.sync if ci % 2 == 0 else nc.gpsimd
                    if ci < 3:
                        q.dma_start(out.ap()[b, c0 + 1:c1 + 1, :],
                                    reg[0:cs])
                    else:
                        q.dma_start(out.ap()[b, c0 + 1:G, :],
                                    reg[0:cs - 1])
                        nc.sync.dma_start(out.ap()[b, 0:1, :],
                                          reg[cs - 1:cs])

            def emit_stream(units, aux, lags=(0, 1, 2, 3), carry_in=(),
                            carry=False):
                """Software-pipelined emission: unit u's phase p is
                emitted at step u + lags[p]. aux closures inject at step
                indices. With carry=True the trailing phases are returned
                for interleaving into the next stream."""
                n = len(units)
                maxlag = lags[-1]
                pend = list(carry_in)
                for i in range(n + (0 if carry else maxlag)):
                    for f in aux.get(i, ()):
                        f()
                    for p, lag in enumerate(lags):
                        u = i - lag
                        if 0 <= u < n:
                            units[u][p]()
                    while pend and pend[0][0] <= i:
                        pend.pop(0)[1]()
                if carry:
                    rest = []
                    for p, lag in enumerate(lags):
                        for u in range(n - lag, n):
                            if 0 <= u < n:
                                rest.append((u + lag - n, units[u][p]))
                    rest.sort(key=lambda t: t[0])
                    return rest
                return []

            loop_cm = (tc.For_i(0, loop_k, 1) if loop_k
                       else contextlib.nullcontext())
            with loop_cm:
                st0, st1 = {}, {}
                proj_dma(0, st0)
                warm = psp.tile([128, 2, 512], f32, tag="sc", bufs=3,
                                name="warm")
                for _ in range(60):
                    nc.tensor.matmul(warm[0:16, 0, 0:64], wz[:, 0:16],
                                     wz[:, 0:64], skip_group_check=True)
                proj_qkk(0, st0)
                proj_qpqd(0, st0, "qp")
                proj_qpqd(0, st0, "qd")
                proj_v(0, st0)
                proj_vcopy(0, st0)
                u0 = stream_units(0, st0)
                carry = emit_stream(u0, {
                    # quad-0 (heads 0-3) attn@V completes at step 16+3
                    20: [lambda: run_gen(tail_norm_q(0, st0, 0))],
                    22: [lambda: proj_dma(1, st1)],
                    23: [lambda: proj_qkk(1, st1, 0)],
                    24: [lambda: proj_qkk(1, st1, 1)],
                    26: [lambda: proj_qpqd(1, st1, "qp")],
                    27: [lambda: proj_qpqd(1, st1, "qd")],
                    28: [lambda: proj_v(1, st1)],
                    29: [lambda: proj_vcopy(1, st1)],
                }, carry=True)
                u1 = stream_units(1, st1)
                emit_stream(u1, {
                    # the carried av of u0-unit-31 (head 7 chunk 3) fires at
                    # the END of step 2, so the H1 tail must come at >= 3
                    3: [lambda: run_gen(tail_norm_q(0, st0, 1))],
                    5: [lambda: tail_final_mms(0, st0, 0)],
                    6: [lambda: tail_final_mms(0, st0, 1)],
                    7: [lambda: tail_final_out(0, st0)],
                    20: [lambda: run_gen(tail_norm_q(1, st1, 0))],
                }, carry_in=carry)
                # terminal: interleave the normalize halves with the final
                # matmuls whose output chunks each half covers
                gen = tail_norm_q(1, st1, 1)
                next(gen)                        # cols 0:256 (chunks 0, 1)
                tail_final_mms(1, st1, 0, (0, 1))
                tail_final_mms(1, st1, 1, (0, 1))
                run_gen(gen)                     # cols 256:501 (chunks 2, 3)
                tail_final_mms(1, st1, 0, (2, 3))
                tail_final_mms(1, st1, 1, (2, 3))
                tail_final_out(1, st1)

    nc.compile()
    return nc


def _prep_weights(W_query, W_key, W_val, W1, W2, W3, W4, W_out):
    nf = 0.25  # 1/sqrt(16), exact power of two
    stack = lambda w: np.ascontiguousarray(
        np.asarray(w, np.float32).transpose(1, 0, 2).reshape(D, D))
    wq = stack(W_query) * nf
    wk = stack(W_key)
    mask = np.zeros((1, D), np.float32)
    for h in range(H):
        if h % 2 == 0:
            mask[0, h * KD:(h + 1) * KD] = 1.0
    wke = wk * mask
    wko = wk * (1.0 - mask)
    wo = np.asarray(W_out, np.float32)
    # W_out rows interleaved into 32-row groups: rows 32j+v hold head
    # (quad*4+j) vector v, rows 32j+16.. (denominator rows) are zero
    wo_pad = np.zeros((2, D, D), np.float32)
    for quad in range(2):
        for j in range(4):
            wo_pad[quad, 32 * j:32 * j + KD] = wo[quad * 4 + j]
    # sel[p, p'] = 1 iff p is the denominator row of p's 32-group
    sel = np.zeros((D, D), np.float32)
    for p2 in range(D):
        sel[32 * (p2 // 32) + 16, p2] = 1.0
    wall = np.concatenate(
        [wq, wke, wko, stack(W1) * nf, stack(W2) * nf, stack(W3) * nf,
         stack(W4) * nf, stack(W_val)], axis=1)
    wobf = np.concatenate([wo_pad[0], wo_pad[1]],
                          axis=1).astype(ml_dtypes.bfloat16)
    return {"wall": np.ascontiguousarray(wall.astype(ml_dtypes.bfloat16)),
            "wobf": np.ascontiguousarray(wobf),
            "sel": np.ascontiguousarray(sel.astype(ml_dtypes.bfloat16))}


def prep_in_maps(inputs):
    """Full harness inputs -> per-core in_maps for run_bass_kernel_spmd."""
    w = _prep_weights(inputs["W_query"], inputs["W_key"], inputs["W_val"],
                      inputs["W1_query"], inputs["W2_query"],
                      inputs["W3_query"], inputs["W4_query"],
                      inputs["W_out"])
    q = np.asarray(inputs["q"], np.float32)
    hTr = np.ascontiguousarray(q[:, ROT, :].transpose(0, 2, 1))
    h_hi = hTr.astype(ml_dtypes.bfloat16)
    h_lo = (hTr - h_hi.astype(np.float32)).astype(ml_dtypes.bfloat16)
    hsplit = np.ascontiguousarray(np.stack([h_hi, h_lo], axis=1))
    return [dict(w, hq=hsplit[BPC * c:BPC * (c + 1)]) for c in range(NCORES)]


def _numpy_fallback(q, W_query, W_key, W_val, W1, W2, W3, W4, W_out,
                    n_pick, n_delivery):
    """Pure-numpy reference for unexpected n_pick/n_delivery (not used for
    the standard 250/250 problem)."""
    h = np.asarray(q, np.float64)
    Bq, Gq, _ = h.shape
    nf = 1.0 / math.sqrt(KD)
    NEG = -np.inf
    proj = lambda x, W: np.einsum("bnd,hdk->hbnk", x, np.asarray(W, np.float64))
    sc = lambda Q, K: nf * np.einsum("hbqk,hbgk->hbqg", Q, K)
    zm = lambda c: np.where(c == 0, NEG, c)
    Q, K, V = proj(h, W_query), proj(h, W_key), proj(h, W_val)
    comp = sc(Q, K)
    hp, hd = h[:, 1:1 + n_pick], h[:, 1 + n_pick:]
    Kp, Vp = proj(hp, W_key), proj(hp, W_val)
    Kd, Vd = proj(hd, W_key), proj(hd, W_val)
    c_pp = zm(sc(proj(hp, W1), Kp))
    c_pd = zm(sc(proj(hp, W2), Kd))
    c_dp = zm(sc(proj(hd, W3), Kp))
    c_dd = zm(sc(proj(hd, W4), Kd))

    def place(blk, r0):
        full = np.full((H, Bq, Gq, blk.shape[3]), NEG)
        full[:, :, r0:r0 + blk.shape[2], :] = blk
        return full

    md = hd.shape[1]
    cf = np.concatenate([comp, place(c_pp, 1), place(c_pd, 1),
                         place(c_dd, Gq - md), place(c_dp, Gq - md)], axis=-1)
    cf -= cf.max(axis=-1, keepdims=True)
    e = np.exp(cf)
    attn = e / e.sum(axis=-1, keepdims=True)
    g, mp = Gq, n_pick
    heads = np.einsum("hbqg,hbgv->hbqv", attn[..., :g], V)
    heads += np.einsum("hbqp,hbpv->hbqv", attn[..., g:g + mp], Vp)
    heads += np.einsum("hbqd,hbdv->hbqv", attn[..., g + mp:g + mp + md], Vd)
    heads += np.einsum("hbqd,hbdv->hbqv",
                       attn[..., g + mp + md:g + mp + 2 * md], Vd)
    heads += np.einsum("hbqp,hbpv->hbqv", attn[..., g + mp + 2 * md:], Vp)
    return np.einsum("hbqv,hve->bqe", heads,
                     np.asarray(W_out, np.float64)).astype(np.float32)


def kernel(q, W_query, W_key, W_val, W1_query, W2_query, W3_query, W4_query,
           W_out, n_pick, n_delivery):
    np_, nd_ = int(n_pick), int(n_delivery)
    q = np.asarray(q, np.float32)
    if np_ != NP or nd_ != ND or q.shape != (B, G, D):
        return _numpy_fallback(q, W_query, W_key, W_val, W1_query, W2_query,
                               W3_query, W4_query, W_out, np_, nd_)

    from concourse import bass_utils

    if "nc" not in _CACHE:
        _CACHE["nc"] = _build_nc()
    nc = _CACHE["nc"]

    in_maps = prep_in_maps(dict(
        q=q, W_query=W_query, W_key=W_key, W_val=W_val, W1_query=W1_query,
        W2_query=W2_query, W3_query=W3_query, W4_query=W4_query,
        W_out=W_out))
    res = bass_utils.run_bass_kernel_spmd(nc, in_maps,
                                          core_ids=list(range(NCORES)))
    return np.concatenate([r["out"] for r in res.results], axis=0)


# revision 33
# speedup vs baseline: 1.2225x; 1.2225x over previous
"""Trainium2 Bass kernel for the pickup/delivery heterogeneous MHA module.

Shapes (hardcoded): q (16, 501, 128) f32, 8 heads, key dim 16,
n_pick = n_delivery = 250, G = 1 + 250 + 250 = 501.

Sharding: data parallel over batch — 2 batches per core on 8 cores.
"""

import sys

for _p in ("/opt/trn_rl_repo", "/root/.axon_site/_ro/trn_rl_repo"):
    if _p not in sys.path:
        sys.path.insert(0, _p)

import math

import ml_dtypes
import numpy as np

B, G, D, H, KD = 16, 501, 128, 8, 16
NP = ND = 250
NCORES = 8
BPC = B // NCORES  # batches per core
F32 = None  # set after imports
BF16 = None

# rotated g order: [picks (g 1..251), delivs (g 251..501), depot (g 0)]
ROT = np.concatenate([np.arange(1, G), [0]])

# main g-chunks in rotated coords (partition-tiles of the key/value axis)
CHUNKS_MAIN = [(0, 128), (128, 250), (250, 378), (378, 501)]
# score slot stream: 4 main chunks + 2 pick-block + 2 deliv-block chunks
# (kind, c0, c1): kind m = vs full QT (N=501), p/d = vs QP/QD (N=500)
SLOTS = [
    ("m", 0, 128), ("m", 128, 250), ("m", 250, 378), ("m", 378, 501),
    ("p", 0, 128), ("p", 128, 250), ("d", 250, 378), ("d", 378, 500),
]
# stream phases: 8 units each; a unit = one chunk x one head pair. Pairs
# hit different PE row groups (scores) and col groups (attn@V); quad 0
# (heads 0-3) finishes at unit 16 so its tail overlaps quad 1's stream.
PHASE_HEADS = [(0, 2), (1, 3), (4, 6), (5, 7)]
# vext chunk index holding rows [c0, c1) (rotated)
VCHUNK = {0: 0, 128: 1, 250: 2, 378: 3}

_CACHE = {}


def _build_nc(loop_k=0):
    """loop_k=0: normal kernel. loop_k>0: wrap the body in a device-side
    For_i loop of loop_k iterations (benchmarking only)."""
    import contextlib

    import concourse.bacc as bacc
    import concourse.mybir as mybir
    import concourse.tile as tile

    f32 = mybir.dt.float32
    bf16 = mybir.dt.bfloat16
    EXP = mybir.ActivationFunctionType.Exp

    nc = bacc.Bacc("TRN2", target_bir_lowering=False, debug=False,
                   num_devices=NCORES)

    # hq carries a hi/lo bf16 split of q^T so projections can run as two
    # accumulating bf16 matmuls (2 cyc/row) instead of fp32 (4 cyc/row)
    hq = nc.dram_tensor("hq", [BPC, 2, D, G], bf16, kind="ExternalInput")
    # all bf16 weights packed into one DMA: wq wke wko w1 w2 w3 w4 wv
    wnames = ["wq", "wke", "wko", "w1", "w2", "w3", "w4", "wv"]
    walldr = nc.dram_tensor("wall", [D, len(wnames) * D], bf16,
                            kind="ExternalInput")
    wobfdr = nc.dram_tensor("wobf", [D, 2 * D], bf16, kind="ExternalInput")
    seldr = nc.dram_tensor("sel", [D, D], f32, kind="ExternalInput")
    out = nc.dram_tensor("out", [BPC, G, D], f32, kind="ExternalOutput")

    with tile.TileContext(nc) as tc:
        with (
            tc.tile_pool(name="const", bufs=1) as constp,
            tc.tile_pool(name="perb", bufs=2) as perb,
            tc.tile_pool(name="vext", bufs=2) as vextp,
            tc.tile_pool(name="expp", bufs=6) as expp,
            tc.tile_pool(name="tmp", bufs=3) as tmpp,
            tc.tile_pool(name="outp", bufs=3) as outp,
            tc.tile_pool(name="ps", bufs=2, space="PSUM") as psp,
            tc.tile_pool(name="hps", bufs=2, space="PSUM") as hpsp,
        ):
            wall = constp.tile([D, len(wnames) * D], bf16, name="wall")
            nc.sync.dma_start(wall[:], walldr.ap())
            wsb = {n: wall[:, i * D:(i + 1) * D]
                   for i, n in enumerate(wnames)}
            # sel/wobf are needed late; keep them off the critical DMA queue
            sel_sb = constp.tile([D, D], f32, name="sel_sb")
            nc.gpsimd.dma_start(sel_sb[:], seldr.ap())
            wobf = constp.tile([D, 2 * D], bf16, name="wobf")
            nc.gpsimd.dma_start(wobf[:], wobfdr.ap())
            wo0_sb = wobf[:, 0:D]
            wo1_sb = wobf[:, D:2 * D]

            # warm the ACT exp table so its one-time load overlaps the
            # initial weight DMAs instead of stalling the first real exp
            wtile = constp.tile([1, 4], f32, name="wtile")
            nc.gpsimd.memset(wtile[:], 0.0)
            wtile2 = constp.tile([1, 4], f32, name="wtile2")
            nc.scalar.activation(wtile2[:], wtile[:], EXP)
            # dummy matmuls keep PE busy during the input DMA wait so the
            # HAM clock-gate is released (2.4 GHz) before the projections
            wz = constp.tile([D, 64], bf16, name="wz")
            nc.gpsimd.memset(wz[:], 0.0)

            def proj_dma(b, st):
                # hi half first: the first projection matmul only needs
                # hT[:, 0, :], so it can start while the lo half transfers
                hT = perb.tile([D, 2, G], bf16, name="hT")
                src = hq.ap()[b].transpose([1, 0, 2])
                nc.sync.dma_start(hT[:, 0, :], src[:, 0, :])
                nc.sync.dma_start(hT[:, 1, :], src[:, 1, :])
                st["hT"] = hT

            def pmm(dst, w, st, col0, col1, first, last=True):
                """projection matmul: accumulate W.T @ (h_hi + h_lo)."""
                hT = st["hT"]
                nc.tensor.matmul(dst, w, hT[:, 0, col0:col1],
                                 start=first, stop=False,
                                 skip_group_check=True)
                nc.tensor.matmul(dst, w, hT[:, 1, col0:col1],
                                 start=False, stop=last,
                                 skip_group_check=True)

            def proj_p1(b, st, part=2):
                """QT | KTe (bf16 hi/lo, all heads stacked: M = 128).
                part: 0 = QT only, 1 = KTe only, 2 = both."""
                if part in (0, 2):
                    qt_ps = psp.tile([128, 512], f32, tag="aux", bufs=2,
                                     name="qt_ps")
                    pmm(qt_ps[:, 0:G], wsb["wq"], st, 0, G, True)
                    qt = perb.tile([D, G], bf16, name="qt")
                    nc.vector.tensor_copy(qt[:], qt_ps[:, 0:G])
                    st["qt"] = qt
                if part in (1, 2):
                    kte_ps = psp.tile([128, 512], f32, tag="aux", bufs=2,
                                      name="kte_ps")
                    pmm(kte_ps[:, 0:G], wsb["wke"], st, 0, G, True)
                    kte = perb.tile([D, G], bf16, name="kte")
                    nc.vector.tensor_copy(kte[:], kte_ps[:, 0:G])
                    st["kte"] = kte

            def proj_v(b, st, half):
                """V-chunk projections, 4 x 128 cols in one aux bank."""
                hT = st["hT"]
                if half == 0:
                    st["v_ps"] = psp.tile([128, 512], f32, tag="aux",
                                          bufs=2, name="v_ps")
                v_ps = st["v_ps"]
                for ci in (2 * half, 2 * half + 1):
                    c0, c1 = CHUNKS_MAIN[ci]
                    for i in range(2):
                        nc.tensor.matmul(
                            v_ps[0:c1 - c0, 128 * ci:128 * ci + 128],
                            hT[:, i, c0:c1], wsb["wv"],
                            start=(ci == 0 and i == 0),
                            stop=(ci == 3 and i == 1),
                            skip_group_check=True)

            def proj_vcopy(b, st):
                """V_ext tiles: per chunk [128, 8 heads * (16 V + 16
                ones)]; the ones columns make the attn@V matmuls
                accumulate softmax denominators for free."""
                v_ps = st["v_ps"]
                vext = []
                for ci, (c0, c1) in enumerate(CHUNKS_MAIN):
                    cs = c1 - c0
                    vt = vextp.tile([128, 256], bf16, tag=f"v{ci}",
                                    name=f"vext{ci}")
                    vv = vt.rearrange("p (h w) -> p h w", h=H)
                    nc.gpsimd.memset(vv[:, :, 16:32], 1.0)
                    src = v_ps[0:cs, 128 * ci:128 * ci + 128]
                    nc.vector.tensor_copy(
                        vv[0:cs, :, 0:16],
                        src.rearrange("p (h v) -> p h v", h=H))
                    vext.append(vt)
                st["vext"] = vext

            def proj_qp(b, st):
                qp_ps = psp.tile([128, 512], f32, tag="aux", bufs=2,
                                 name="qp_ps")
                pmm(qp_ps[:, 0:250], wsb["w1"], st, 0, 250, True,
                    last=False)
                pmm(qp_ps[:, 250:500], wsb["w3"], st, 250, 500, False)
                qp = perb.tile([D, 500], bf16, name="qp")
                nc.vector.tensor_copy(qp[:], qp_ps[:, 0:500])
                st["qp"] = qp

            def proj_qd(b, st):
                qd_ps = psp.tile([128, 512], f32, tag="aux", bufs=2,
                                 name="qd_ps")
                pmm(qd_ps[:, 0:250], wsb["w2"], st, 0, 250, True,
                    last=False)
                pmm(qd_ps[:, 250:500], wsb["w4"], st, 250, 500, False)
                qd = perb.tile([D, 500], bf16, name="qd")
                nc.vector.tensor_copy(qd[:], qd_ps[:, 0:500])
                st["qd"] = qd

            def proj_kto(b, st):
                kto_ps = psp.tile([128, 512], f32, tag="aux", bufs=2,
                                  name="kto_ps")
                pmm(kto_ps[:, 0:G], wsb["wko"], st, 0, G, True)
                kto = perb.tile([D, G], bf16, name="kto")
                nc.vector.tensor_copy(kto[:], kto_ps[:, 0:G])
                st["kto"] = kto

            def stream_units(b, st):
                """32 units: each = 2 score matmuls -> exp -> 2 attn@V.
                Units 0..16 cover quad 0 (phases A: heads 0/2, B: 1/3),
                units 16..32 quad 1."""
                H0 = hpsp.tile([128, 512], f32, tag="H", name="H0")
                H1 = hpsp.tile([128, 512], f32, tag="H", name="H1")
                st["Hs"] = (H0, H1)

                def unit(u):
                    ph, s = u // 8, u % 8
                    heads = PHASE_HEADS[ph]
                    kind, c0, c1 = SLOTS[s]
                    cs = c1 - c0
                    pend = []

                    def emit_scores():
                        cur = psp.tile([128, 2, 512], f32, tag="sc",
                                       name="sc")
                        n = G if kind == "m" else 500
                        for sub, h in enumerate(heads):
                            p, par = h // 2, h % 2
                            kt = st["kte"] if par == 0 else st["kto"]
                            rhs = (st["qt"] if kind == "m" else
                                   st["qp"] if kind == "p" else st["qd"])
                            rhs = rhs[32 * p:32 * p + 32, :]
                            nc.tensor.matmul(
                                cur[0:cs, sub, 0:rhs.shape[1]],
                                kt[32 * p:32 * p + 32, c0:c1], rhs,
                                tile_position=(32 * p, 0))
                        ex = expp.tile([128, 2, 501], bf16, name="ex")
                        nc.scalar.activation(ex[0:cs, :, 0:n],
                                             cur[0:cs, :, 0:n], EXP)
                        pend.append(ex)

                    def emit_avs():
                        ex = pend.pop()
                        n = G if kind == "m" else 500
                        vt = st["vext"][VCHUNK[c0]]
                        Hq = H0 if ph < 2 else H1
                        for sub, h in enumerate(heads):
                            cg = 32 * (h % 4)
                            # PSUM pending-zero is per partition range:
                            # each head's col-group needs its own start
                            nc.tensor.matmul(
                                Hq[cg:cg + 32, 0:n],
                                vt[0:cs, 32 * h:32 * h + 32],
                                ex[0:cs, sub, 0:n],
                                start=(s == 0), stop=(s == len(SLOTS) - 1),
                                tile_position=(0, cg),
                                skip_group_check=True)
                    return emit_scores, emit_avs

                return [unit(u) for u in range(4 * len(SLOTS))]

            def tail_norm_q(b, st, quad):
                """normalize one quad by its softmax denominators. The
                sel-matmul broadcasts each 32-group's denominator row to
                the whole group (engine partition accesses must be
                32-aligned, so row 32j+16 cannot be sliced directly).
                Column-split in two halves to pipeline the serial chain."""
                Hq = st["Hs"][quad]
                hsb = tmpp.tile([D, G], f32, tag="hsb", name="hsb")
                denb = psp.tile([128, 512], f32, tag="aux", bufs=2,
                                name="denb")
                rcb = tmpp.tile([D, G], f32, tag="rcb", name="rcb")
                hn = perb.tile([D, G], bf16, name=f"hn{quad}")
                st[f"hn{quad}"] = hn
                for c0, c1 in ((0, 256), (256, G)):
                    nc.vector.tensor_copy(hsb[:, c0:c1], Hq[:, c0:c1])
                    nc.tensor.matmul(denb[:, c0:c1], sel_sb,
                                     hsb[:, c0:c1], start=(c0 == 0),
                                     stop=(c0 != 0), skip_group_check=True)
                    nc.vector.reciprocal_approx_fast(rcb[:, c0:c1],
                                                     denb[:, c0:c1])
                    nc.vector.tensor_mul(hn[:, c0:c1], hsb[:, c0:c1],
                                         rcb[:, c0:c1])
                    yield

            def run_gen(gen):
                for _ in gen:
                    pass

            def tail_final_mms(b, st, quad, cis=(0, 1, 2, 3)):
                """out = sum_h headsT_h @ W_out_h, accumulated across the
                two quads into two aux psum banks (2 chunk regions each)."""
                hn = st[f"hn{quad}"]
                wo = wo0_sb if quad == 0 else wo1_sb
                if "ops" not in st:
                    st["ops"] = [
                        psp.tile([128, 512], f32, tag="aux", bufs=2,
                                 name="ops_a"),
                        psp.tile([128, 512], f32, tag="aux", bufs=2,
                                 name="ops_b")]
                for ci in cis:
                    c0, c1 = CHUNKS_MAIN[ci]
                    # one start=True per BANK (ci even); the odd region
                    # relies on the bank-wide pending-zero from it
                    nc.tensor.matmul(
                        st["ops"][ci // 2][0:c1 - c0,
                                           128 * (ci % 2):128 * (ci % 2) + 128],
                        hn[:, c0:c1], wo,
                        start=(quad == 0 and ci % 2 == 0),
                        stop=(quad == 1 and ci % 2 == 1),
                        skip_group_check=True)

            def tail_final_out(b, st):
                osb = outp.tile([128, 2, 512], f32, name="osb")
                for ci, (c0, c1) in enumerate(CHUNKS_MAIN):
                    cs = c1 - c0
                    sl = slice(128 * (ci % 2), 128 * (ci % 2) + 128)
                    nc.vector.tensor_copy(osb[0:cs, ci // 2, sl],
                                          st["ops"][ci // 2][0:cs, sl])
                    reg = osb[:, ci // 2, sl]
                    if ci < 3:
                        nc.sync.dma_start(out.ap()[b, c0 + 1:c1 + 1, :],
                                          reg[0:cs])
                    else:
                        nc.sync.dma_start(out.ap()[b, c0 + 1:G, :],
                                          reg[0:cs - 1])
                        nc.sync.dma_start(out.ap()[b, 0:1, :],
                                          reg[cs - 1:cs])

            def emit_stream(units, aux, depth=2, carry_in=(), carry=False):
                """Software-pipelined emission: unit u's attn@V matmuls
                are emitted after unit u+depth's scores, so PE stays ahead
                of the exp engine. aux closures inject at unit indices.
                With carry=True the last `depth` AVs are returned so they
                can be interleaved into the next stream's start."""
                avq = list(carry_in)
                for i, (sc, av) in enumerate(units):
                    for f in aux.get(i, ()):
                        f()
                    sc()
                    avq.append(av)
                    if len(avq) > depth:
                        avq.pop(0)()
                if carry:
                    return avq
                for av in avq:
                    av()
                return []

            loop_cm = (tc.For_i(0, loop_k, 1) if loop_k
                       else contextlib.nullcontext())
            with loop_cm:
                # pipeline the two batches: b1's projections, b0's per-quad
                # tails, and the stream are interleaved in ~1us pieces so
                # the exp engine (the bottleneck) never runs dry
                st0, st1 = {}, {}
                proj_dma(0, st0)
                warm = psp.tile([128, 512], f32, tag="aux", bufs=2,
                                name="warm")
                for _ in range(24):
                    nc.tensor.matmul(warm[0:16, 0:64], wz[:, 0:16],
                                     wz[:, 0:64], skip_group_check=True)
                proj_p1(0, st0)
                u0 = stream_units(0, st0)
                carry = emit_stream(u0, {
                    1: [lambda: proj_v(0, st0, 0)],
                    2: [lambda: proj_v(0, st0, 1),
                        lambda: proj_vcopy(0, st0)],
                    3: [lambda: proj_qp(0, st0)],
                    4: [lambda: proj_qd(0, st0)],
                    5: [lambda: proj_kto(0, st0)],
                    # unit 15's deferred attn@V (last write into H0) pops
                    # at boundary 17, so quad-0 tails go at 18+
                    18: [lambda: run_gen(tail_norm_q(0, st0, 0))],
                    21: [lambda: proj_dma(1, st1)],
                    23: [lambda: proj_p1(1, st1, 0)],
                    24: [lambda: proj_p1(1, st1, 1)],
                    25: [lambda: proj_v(1, st1, 0)],
                    26: [lambda: proj_v(1, st1, 1)],
                    27: [lambda: proj_vcopy(1, st1)],
                    28: [lambda: proj_qp(1, st1)],
                    29: [lambda: proj_qd(1, st1)],
                    30: [lambda: proj_kto(1, st1)],
                }, carry=True)
                u1 = stream_units(1, st1)
                emit_stream(u1, {
                    2: [lambda: run_gen(tail_norm_q(0, st0, 1))],
                    4: [lambda: tail_final_mms(0, st0, 0)],
                    5: [lambda: tail_final_mms(0, st0, 1)],
                    6: [lambda: tail_final_out(0, st0)],
                    18: [lambda: run_gen(tail_norm_q(1, st1, 0))],
                }, carry_in=carry)
                # terminal: interleave the normalize halves with the final
                # matmuls whose output chunks each half covers
                gen = tail_norm_q(1, st1, 1)
                next(gen)                        # cols 0:256 (chunks 0, 1)
                tail_final_mms(1, st1, 0, (0, 1))
                tail_final_mms(1, st1, 1, (0, 1))
                run_gen(gen)                     # cols 256:501 (chunks 2, 3)
                tail_final_mms(1, st1, 0, (2, 3))
                tail_final_mms(1, st1, 1, (2, 3))
                tail_final_out(1, st1)

    nc.compile()
    return nc


def _prep_weights(W_query, W_key, W_val, W1, W2, W3, W4, W_out):
    nf = 0.25  # 1/sqrt(16), exact power of two
    stack = lambda w: np.ascontiguousarray(
        np.asarray(w, np.float32).transpose(1, 0, 2).reshape(D, D))
    wq = stack(W_query) * nf
    wk = stack(W_key)
    mask = np.zeros((1, D), np.float32)
    for h in range(H):
        if h % 2 == 0:
            mask[0, h * KD:(h + 1) * KD] = 1.0
    wke = wk * mask
    wko = wk * (1.0 - mask)
    wo = np.asarray(W_out, np.float32)
    # W_out rows interleaved into 32-row groups: rows 32j+v hold head
    # (quad*4+j) vector v, rows 32j+16.. (denominator rows) are zero
    wo_pad = np.zeros((2, D, D), np.float32)
    for quad in range(2):
        for j in range(4):
            wo_pad[quad, 32 * j:32 * j + KD] = wo[quad * 4 + j]
    # sel[p, p'] = 1 iff p is the denominator row of p's 32-group
    sel = np.zeros((D, D), np.float32)
    for p2 in range(D):
        sel[32 * (p2 // 32) + 16, p2] = 1.0
    wall = np.concatenate(
        [wq, wke, wko, stack(W1) * nf, stack(W2) * nf, stack(W3) * nf,
         stack(W4) * nf, stack(W_val)], axis=1)
    wobf = np.concatenate([wo_pad[0], wo_pad[1]],
                          axis=1).astype(ml_dtypes.bfloat16)
    return {"wall": np.ascontiguousarray(wall.astype(ml_dtypes.bfloat16)),
            "wobf": np.ascontiguousarray(wobf),
            "sel": np.ascontiguousarray(sel)}


def prep_in_maps(inputs):
    """Full harness inputs -> per-core in_maps for run_bass_kernel_spmd."""
    w = _prep_weights(inputs["W_query"], inputs["W_key"], inputs["W_val"],
                      inputs["W1_query"], inputs["W2_query"],
                      inputs["W3_query"], inputs["W4_query"],
                      inputs["W_out"])
    q = np.asarray(inputs["q"], np.float32)
    hTr = np.ascontiguousarray(q[:, ROT, :].transpose(0, 2, 1))
    h_hi = hTr.astype(ml_dtypes.bfloat16)
    h_lo = (hTr - h_hi.astype(np.float32)).astype(ml_dtypes.bfloat16)
    hsplit = np.ascontiguousarray(np.stack([h_hi, h_lo], axis=1))
    return [dict(w, hq=hsplit[BPC * c:BPC * (c + 1)]) for c in range(NCORES)]


def _numpy_fallback(q, W_query, W_key, W_val, W1, W2, W3, W4, W_out,
                    n_pick, n_delivery):
    """Pure-numpy reference for unexpected n_pick/n_delivery (not used for
    the standard 250/250 problem)."""
    h = np.asarray(q, np.float64)
    Bq, Gq, _ = h.shape
    nf = 1.0 / math.sqrt(KD)
    NEG = -np.inf
    proj = lambda x, W: np.einsum("bnd,hdk->hbnk", x, np.asarray(W, np.float64))
    sc = lambda Q, K: nf * np.einsum("hbqk,hbgk->hbqg", Q, K)
    zm = lambda c: np.where(c == 0, NEG, c)
    Q, K, V = proj(h, W_query), proj(h, W_key), proj(h, W_val)
    comp = sc(Q, K)
    hp, hd = h[:, 1:1 + n_pick], h[:, 1 + n_pick:]
    Kp, Vp = proj(hp, W_key), proj(hp, W_val)
    Kd, Vd = proj(hd, W_key), proj(hd, W_val)
    c_pp = zm(sc(proj(hp, W1), Kp))
    c_pd = zm(sc(proj(hp, W2), Kd))
    c_dp = zm(sc(proj(hd, W3), Kp))
    c_dd = zm(sc(proj(hd, W4), Kd))

    def place(blk, r0):
        full = np.full((H, Bq, Gq, blk.shape[3]), NEG)
        full[:, :, r0:r0 + blk.shape[2], :] = blk
        return full

    md = hd.shape[1]
    cf = np.concatenate([comp, place(c_pp, 1), place(c_pd, 1),
                         place(c_dd, Gq - md), place(c_dp, Gq - md)], axis=-1)
    cf -= cf.max(axis=-1, keepdims=True)
    e = np.exp(cf)
    attn = e / e.sum(axis=-1, keepdims=True)
    g, mp = Gq, n_pick
    heads = np.einsum("hbqg,hbgv->hbqv", attn[..., :g], V)
    heads += np.einsum("hbqp,hbpv->hbqv", attn[..., g:g + mp], Vp)
    heads += np.einsum("hbqd,hbdv->hbqv", attn[..., g + mp:g + mp + md], Vd)
    heads += np.einsum("hbqd,hbdv->hbqv",
                       attn[..., g + mp + md:g + mp + 2 * md], Vd)
    heads += np.einsum("hbqp,hbpv->hbqv", attn[..., g + mp + 2 * md:], Vp)
    return np.einsum("hbqv,hve->bqe", heads,
                     np.asarray(W_out, np.float64)).astype(np.float32)


def kernel(q, W_query, W_key, W_val, W1_query, W2_query, W3_query, W4_query,
           W_out, n_pick, n_delivery):
    np_, nd_ = int(n_pick), int(n_delivery)
    q = np.asarray(q, np.float32)
    if np_ != NP or nd_ != ND or q.shape != (B, G, D):
        return _numpy_fallback(q, W_query, W_key, W_val, W1_query, W2_query,
                               W3_query, W4_query, W_out, np_, nd_)

    from concourse import bass_utils

    if "nc" not in _CACHE:
        _CACHE["nc"] = _build_nc()
    nc = _CACHE["nc"]

    w = _prep_weights(W_query, W_key, W_val, W1_query, W2_query, W3_query,
                      W4_query, W_out)
    # host layout: rotate g axis (picks, delivs, depot), transpose to
    # [b, d, g], and split into bf16 hi + lo residual so the device
    # projections run as two accumulating bf16 matmuls
    hTr = np.ascontiguousarray(q[:, ROT, :].transpose(0, 2, 1))
    h_hi = hTr.astype(ml_dtypes.bfloat16)
    h_lo = (hTr - h_hi.astype(np.float32)).astype(ml_dtypes.bfloat16)
    hsplit = np.ascontiguousarray(np.stack([h_hi, h_lo], axis=1))

    in_maps = [dict(w, hq=hsplit[BPC * c:BPC * (c + 1)])
               for c in range(NCORES)]
    res = bass_utils.run_bass_kernel_spmd(nc, in_maps,
                                          core_ids=list(range(NCORES)))
    return np.concatenate([r["out"] for r in res.results], axis=0)



# revision 35
# speedup vs baseline: 1.2417x; 1.0157x over previous
"""Trainium2 Bass kernel for the pickup/delivery heterogeneous MHA module.

Shapes (hardcoded): q (16, 501, 128) f32, 8 heads, key dim 16,
n_pick = n_delivery = 250, G = 1 + 250 + 250 = 501.

Sharding: data parallel over batch — 2 batches per core on 8 cores.
"""

import sys

for _p in ("/opt/trn_rl_repo", "/root/.axon_site/_ro/trn_rl_repo"):
    if _p not in sys.path:
        sys.path.insert(0, _p)

import math

import ml_dtypes
import numpy as np

B, G, D, H, KD = 16, 501, 128, 8, 16
NP = ND = 250
NCORES = 8
BPC = B // NCORES  # batches per core
F32 = None  # set after imports
BF16 = None

# rotated g order: [picks (g 1..251), delivs (g 251..501), depot (g 0)]
ROT = np.concatenate([np.arange(1, G), [0]])

# main g-chunks in rotated coords (partition-tiles of the key/value axis)
CHUNKS_MAIN = [(0, 128), (128, 250), (250, 378), (378, 501)]
# score slot stream: 4 main chunks + 2 pick-block + 2 deliv-block chunks
# (kind, c0, c1): kind m = vs full QT (N=501), p/d = vs QP/QD (N=500)
SLOTS = [
    ("m", 0, 128), ("m", 128, 250), ("m", 250, 378), ("m", 378, 501),
    ("p", 0, 128), ("p", 128, 250), ("d", 250, 378), ("d", 378, 500),
]
# stream phases: 8 units each; a unit = one chunk x one head pair. Pairs
# hit different PE row groups (scores) and col groups (attn@V); quad 0
# (heads 0-3) finishes at unit 16 so its tail overlaps quad 1's stream.
PHASE_HEADS = [(0, 2), (1, 3), (4, 6), (5, 7)]
# vext chunk index holding rows [c0, c1) (rotated)
VCHUNK = {0: 0, 128: 1, 250: 2, 378: 3}

_CACHE = {}


def _build_nc(loop_k=0):
    """loop_k=0: normal kernel. loop_k>0: wrap the body in a device-side
    For_i loop of loop_k iterations (benchmarking only)."""
    import contextlib

    import concourse.bacc as bacc
    import concourse.mybir as mybir
    import concourse.tile as tile

    f32 = mybir.dt.float32
    bf16 = mybir.dt.bfloat16
    i16 = mybir.dt.int16
    EXP = mybir.ActivationFunctionType.Exp
    MULT = mybir.AluOpType.mult
    ADD = mybir.AluOpType.add
    # Schraudolph exp -> bf16 bits via one DVE tensor_scalar (HW-validated
    # rel-err cost ~+1e-3 per offloaded head pair; see memory notes)
    SCH_A = 128.0 / math.log(2.0)
    SCH_B = 127.0 * 128.0 - 5.5 + 0.5

    nc = bacc.Bacc("TRN2", target_bir_lowering=False, debug=False,
                   num_devices=NCORES)

    # hq carries a hi/lo bf16 split of q^T so projections can run as two
    # accumulating bf16 matmuls (2 cyc/row) instead of fp32 (4 cyc/row)
    hq = nc.dram_tensor("hq", [BPC, 2, D, G], bf16, kind="ExternalInput")
    # all bf16 weights packed into one DMA: wq wke wko w1 w2 w3 w4 wv
    wnames = ["wq", "wke", "wko", "w1", "w2", "w3", "w4", "wv"]
    walldr = nc.dram_tensor("wall", [D, len(wnames) * D], bf16,
                            kind="ExternalInput")
    wobfdr = nc.dram_tensor("wobf", [D, 2 * D], bf16, kind="ExternalInput")
    seldr = nc.dram_tensor("sel", [D, D], f32, kind="ExternalInput")
    out = nc.dram_tensor("out", [BPC, G, D], f32, kind="ExternalOutput")

    with tile.TileContext(nc) as tc:
        with (
            tc.tile_pool(name="const", bufs=1) as constp,
            tc.tile_pool(name="perb", bufs=2) as perb,
            tc.tile_pool(name="vext", bufs=2) as vextp,
            tc.tile_pool(name="expp", bufs=6) as expp,
            tc.tile_pool(name="tmp", bufs=3) as tmpp,
            tc.tile_pool(name="outp", bufs=3) as outp,
            tc.tile_pool(name="ps", bufs=2, space="PSUM") as psp,
            tc.tile_pool(name="hps", bufs=2, space="PSUM") as hpsp,
        ):
            wall = constp.tile([D, len(wnames) * D], bf16, name="wall")
            nc.sync.dma_start(wall[:], walldr.ap())
            wsb = {n: wall[:, i * D:(i + 1) * D]
                   for i, n in enumerate(wnames)}
            # sel/wobf are needed late; keep them off the critical DMA queue
            sel_sb = constp.tile([D, D], f32, name="sel_sb")
            nc.gpsimd.dma_start(sel_sb[:], seldr.ap())
            wobf = constp.tile([D, 2 * D], bf16, name="wobf")
            nc.gpsimd.dma_start(wobf[:], wobfdr.ap())
            wo0_sb = wobf[:, 0:D]
            wo1_sb = wobf[:, D:2 * D]

            # warm the ACT exp table so its one-time load overlaps the
            # initial weight DMAs instead of stalling the first real exp
            wtile = constp.tile([1, 4], f32, name="wtile")
            nc.gpsimd.memset(wtile[:], 0.0)
            wtile2 = constp.tile([1, 4], f32, name="wtile2")
            nc.scalar.activation(wtile2[:], wtile[:], EXP)
            # dummy matmuls keep PE busy during the input DMA wait so the
            # HAM clock-gate is released (2.4 GHz) before the projections
            wz = constp.tile([D, 64], bf16, name="wz")
            nc.gpsimd.memset(wz[:], 0.0)

            def proj_dma(b, st):
                # hi half first: the first projection matmul only needs
                # hT[:, 0, :], so it can start while the lo half transfers
                hT = perb.tile([D, 2, G], bf16, name="hT")
                src = hq.ap()[b].transpose([1, 0, 2])
                nc.sync.dma_start(hT[:, 0, :], src[:, 0, :])
                nc.sync.dma_start(hT[:, 1, :], src[:, 1, :])
                st["hT"] = hT

            def pmm(dst, w, st, col0, col1, first, last=True):
                """projection matmul: accumulate W.T @ (h_hi + h_lo)."""
                hT = st["hT"]
                nc.tensor.matmul(dst, w, hT[:, 0, col0:col1],
                                 start=first, stop=False,
                                 skip_group_check=True)
                nc.tensor.matmul(dst, w, hT[:, 1, col0:col1],
                                 start=False, stop=last,
                                 skip_group_check=True)

            def proj_p1(b, st, part=2):
                """QT | KTe (bf16 hi/lo, all heads stacked: M = 128).
                part: 0 = QT only, 1 = KTe only, 2 = both."""
                if part in (0, 2):
                    qt_ps = psp.tile([128, 512], f32, tag="aux", bufs=2,
                                     name="qt_ps")
                    pmm(qt_ps[:, 0:G], wsb["wq"], st, 0, G, True)
                    qt = perb.tile([D, G], bf16, name="qt")
                    nc.vector.tensor_copy(qt[:], qt_ps[:, 0:G])
                    st["qt"] = qt
                if part in (1, 2):
                    kte_ps = psp.tile([128, 512], f32, tag="aux", bufs=2,
                                      name="kte_ps")
                    pmm(kte_ps[:, 0:G], wsb["wke"], st, 0, G, True)
                    kte = perb.tile([D, G], bf16, name="kte")
                    nc.vector.tensor_copy(kte[:], kte_ps[:, 0:G])
                    st["kte"] = kte

            def proj_v(b, st, half):
                """V-chunk projections, 4 x 128 cols in one aux bank."""
                hT = st["hT"]
                if half == 0:
                    st["v_ps"] = psp.tile([128, 512], f32, tag="aux",
                                          bufs=2, name="v_ps")
                v_ps = st["v_ps"]
                for ci in (2 * half, 2 * half + 1):
                    c0, c1 = CHUNKS_MAIN[ci]
                    for i in range(2):
                        nc.tensor.matmul(
                            v_ps[0:c1 - c0, 128 * ci:128 * ci + 128],
                            hT[:, i, c0:c1], wsb["wv"],
                            start=(ci == 0 and i == 0),
                            stop=(ci == 3 and i == 1),
                            skip_group_check=True)

            def proj_vcopy(b, st):
                """V_ext tiles: per chunk [128, 8 heads * (16 V + 16
                ones)]; the ones columns make the attn@V matmuls
                accumulate softmax denominators for free."""
                v_ps = st["v_ps"]
                vext = []
                for ci, (c0, c1) in enumerate(CHUNKS_MAIN):
                    cs = c1 - c0
                    vt = vextp.tile([128, 256], bf16, tag=f"v{ci}",
                                    name=f"vext{ci}")
                    vv = vt.rearrange("p (h w) -> p h w", h=H)
                    nc.gpsimd.memset(vv[:, :, 16:32], 1.0)
                    src = v_ps[0:cs, 128 * ci:128 * ci + 128]
                    nc.vector.tensor_copy(
                        vv[0:cs, :, 0:16],
                        src.rearrange("p (h v) -> p h v", h=H))
                    vext.append(vt)
                st["vext"] = vext

            def proj_qp(b, st):
                qp_ps = psp.tile([128, 512], f32, tag="aux", bufs=2,
                                 name="qp_ps")
                pmm(qp_ps[:, 0:250], wsb["w1"], st, 0, 250, True,
                    last=False)
                pmm(qp_ps[:, 250:500], wsb["w3"], st, 250, 500, False)
                qp = perb.tile([D, 500], bf16, name="qp")
                nc.vector.tensor_copy(qp[:], qp_ps[:, 0:500])
                st["qp"] = qp

            def proj_qd(b, st):
                qd_ps = psp.tile([128, 512], f32, tag="aux", bufs=2,
                                 name="qd_ps")
                pmm(qd_ps[:, 0:250], wsb["w2"], st, 0, 250, True,
                    last=False)
                pmm(qd_ps[:, 250:500], wsb["w4"], st, 250, 500, False)
                qd = perb.tile([D, 500], bf16, name="qd")
                nc.vector.tensor_copy(qd[:], qd_ps[:, 0:500])
                st["qd"] = qd

            def proj_kto(b, st):
                kto_ps = psp.tile([128, 512], f32, tag="aux", bufs=2,
                                  name="kto_ps")
                pmm(kto_ps[:, 0:G], wsb["wko"], st, 0, G, True)
                kto = perb.tile([D, G], bf16, name="kto")
                nc.vector.tensor_copy(kto[:], kto_ps[:, 0:G])
                st["kto"] = kto

            def stream_units(b, st):
                """32 units: each = 2 score matmuls -> exp -> 2 attn@V.
                Units 0..16 cover quad 0 (phases A: heads 0/2, B: 1/3),
                units 16..32 quad 1."""
                H0 = hpsp.tile([128, 512], f32, tag="H", name="H0")
                H1 = hpsp.tile([128, 512], f32, tag="H", name="H1")
                st["Hs"] = (H0, H1)

                def unit(u):
                    ph, s = u // 8, u % 8
                    heads = PHASE_HEADS[ph]
                    kind, c0, c1 = SLOTS[s]
                    cs = c1 - c0
                    pend = []

                    def emit_scores():
                        cur = psp.tile([128, 2, 512], f32, tag="sc",
                                       name="sc")
                        n = G if kind == "m" else 500
                        for sub, h in enumerate(heads):
                            p, par = h // 2, h % 2
                            kt = st["kte"] if par == 0 else st["kto"]
                            rhs = (st["qt"] if kind == "m" else
                                   st["qp"] if kind == "p" else st["qd"])
                            rhs = rhs[32 * p:32 * p + 32, :]
                            nc.tensor.matmul(
                                cur[0:cs, sub, 0:rhs.shape[1]],
                                kt[32 * p:32 * p + 32, c0:c1], rhs,
                                tile_position=(32 * p, 0))
                        ex = expp.tile([128, 2, 501], bf16, name="ex")
                        # phase 0 (heads 0/2) exps run as Schraudolph on
                        # DVE to offload the ACT bottleneck; per-pair
                        # assignment keeps each softmax row's exp flavor
                        # consistent so errors cancel in the ratio
                        if ph == 0:
                            nc.vector.tensor_scalar(
                                ex.bitcast(i16)[0:cs, :, 0:n],
                                cur[0:cs, :, 0:n], SCH_A, SCH_B, MULT, ADD)
                        else:
                            nc.scalar.activation(ex[0:cs, :, 0:n],
                                                 cur[0:cs, :, 0:n], EXP)
                        pend.append(ex)

                    def emit_avs():
                        ex = pend.pop()
                        n = G if kind == "m" else 500
                        vt = st["vext"][VCHUNK[c0]]
                        Hq = H0 if ph < 2 else H1
                        for sub, h in enumerate(heads):
                            cg = 32 * (h % 4)
                            # PSUM pending-zero is per partition range:
                            # each head's col-group needs its own start
                            nc.tensor.matmul(
                                Hq[cg:cg + 32, 0:n],
                                vt[0:cs, 32 * h:32 * h + 32],
                                ex[0:cs, sub, 0:n],
                                start=(s == 0), stop=(s == len(SLOTS) - 1),
                                tile_position=(0, cg),
                                skip_group_check=True)
                    return emit_scores, emit_avs

                return [unit(u) for u in range(4 * len(SLOTS))]

            def tail_norm_q(b, st, quad):
                """normalize one quad by its softmax denominators. The
                sel-matmul broadcasts each 32-group's denominator row to
                the whole group (engine partition accesses must be
                32-aligned, so row 32j+16 cannot be sliced directly).
                Column-split in two halves to pipeline the serial chain."""
                Hq = st["Hs"][quad]
                hsb = tmpp.tile([D, G], f32, tag="hsb", name="hsb")
                denb = psp.tile([128, 512], f32, tag="aux", bufs=2,
                                name="denb")
                rcb = tmpp.tile([D, G], f32, tag="rcb", name="rcb")
                hn = perb.tile([D, G], bf16, name=f"hn{quad}")
                st[f"hn{quad}"] = hn
                for c0, c1 in ((0, 256), (256, G)):
                    nc.vector.tensor_copy(hsb[:, c0:c1], Hq[:, c0:c1])
                    nc.tensor.matmul(denb[:, c0:c1], sel_sb,
                                     hsb[:, c0:c1], start=(c0 == 0),
                                     stop=(c0 != 0), skip_group_check=True)
                    nc.vector.reciprocal_approx_fast(rcb[:, c0:c1],
                                                     denb[:, c0:c1])
                    nc.vector.tensor_mul(hn[:, c0:c1], hsb[:, c0:c1],
                                         rcb[:, c0:c1])
                    yield

            def run_gen(gen):
                for _ in gen:
                    pass

            def tail_final_mms(b, st, quad, cis=(0, 1, 2, 3)):
                """out = sum_h headsT_h @ W_out_h, accumulated across the
                two quads into two aux psum banks (2 chunk regions each)."""
                hn = st[f"hn{quad}"]
                wo = wo0_sb if quad == 0 else wo1_sb
                if "ops" not in st:
                    st["ops"] = [
                        psp.tile([128, 512], f32, tag="aux", bufs=2,
                                 name="ops_a"),
                        psp.tile([128, 512], f32, tag="aux", bufs=2,
                                 name="ops_b")]
                for ci in cis:
                    c0, c1 = CHUNKS_MAIN[ci]
                    # one start=True per BANK (ci even); the odd region
                    # relies on the bank-wide pending-zero from it
                    nc.tensor.matmul(
                        st["ops"][ci // 2][0:c1 - c0,
                                           128 * (ci % 2):128 * (ci % 2) + 128],
                        hn[:, c0:c1], wo,
                        start=(quad == 0 and ci % 2 == 0),
                        stop=(quad == 1 and ci % 2 == 1),
                        skip_group_check=True)

            def tail_final_out(b, st):
                osb = outp.tile([128, 2, 512], f32, name="osb")
                for ci, (c0, c1) in enumerate(CHUNKS_MAIN):
                    cs = c1 - c0
                    sl = slice(128 * (ci % 2), 128 * (ci % 2) + 128)
                    nc.vector.tensor_copy(osb[0:cs, ci // 2, sl],
                                          st["ops"][ci // 2][0:cs, sl])
                    reg = osb[:, ci // 2, sl]
                    if ci < 3:
                        nc.sync.dma_start(out.ap()[b, c0 + 1:c1 + 1, :],
                                          reg[0:cs])
                    else:
                        nc.sync.dma_start(out.ap()[b, c0 + 1:G, :],
                                          reg[0:cs - 1])
                        nc.sync.dma_start(out.ap()[b, 0:1, :],
                                          reg[cs - 1:cs])

            def emit_stream(units, aux, depth=2, carry_in=(), carry=False):
                """Software-pipelined emission: unit u's attn@V matmuls
                are emitted after unit u+depth's scores, so PE stays ahead
                of the exp engine. aux closures inject at unit indices.
                With carry=True the last `depth` AVs are returned so they
                can be interleaved into the next stream's start."""
                avq = list(carry_in)
                for i, (sc, av) in enumerate(units):
                    for f in aux.get(i, ()):
                        f()
                    sc()
                    avq.append(av)
                    if len(avq) > depth:
                        avq.pop(0)()
                if carry:
                    return avq
                for av in avq:
                    av()
                return []

            loop_cm = (tc.For_i(0, loop_k, 1) if loop_k
                       else contextlib.nullcontext())
            with loop_cm:
                # pipeline the two batches: b1's projections, b0's per-quad
                # tails, and the stream are interleaved in ~1us pieces so
                # the exp engine (the bottleneck) never runs dry
                st0, st1 = {}, {}
                proj_dma(0, st0)
                warm = psp.tile([128, 512], f32, tag="aux", bufs=2,
                                name="warm")
                for _ in range(24):
                    nc.tensor.matmul(warm[0:16, 0:64], wz[:, 0:16],
                                     wz[:, 0:64], skip_group_check=True)
                proj_p1(0, st0)
                u0 = stream_units(0, st0)
                carry = emit_stream(u0, {
                    1: [lambda: proj_v(0, st0, 0)],
                    2: [lambda: proj_v(0, st0, 1),
                        lambda: proj_vcopy(0, st0)],
                    3: [lambda: proj_qp(0, st0)],
                    4: [lambda: proj_qd(0, st0)],
                    5: [lambda: proj_kto(0, st0)],
                    # unit 15's deferred attn@V (last write into H0) pops
                    # at boundary 17, so quad-0 tails go at 18+
                    18: [lambda: run_gen(tail_norm_q(0, st0, 0))],
                    21: [lambda: proj_dma(1, st1)],
                    23: [lambda: proj_p1(1, st1, 0)],
                    24: [lambda: proj_p1(1, st1, 1)],
                    25: [lambda: proj_v(1, st1, 0)],
                    26: [lambda: proj_v(1, st1, 1)],
                    27: [lambda: proj_vcopy(1, st1)],
                    28: [lambda: proj_qp(1, st1)],
                    29: [lambda: proj_qd(1, st1)],
                    30: [lambda: proj_kto(1, st1)],
                }, carry=True)
                u1 = stream_units(1, st1)
                emit_stream(u1, {
                    2: [lambda: run_gen(tail_norm_q(0, st0, 1))],
                    4: [lambda: tail_final_mms(0, st0, 0)],
                    5: [lambda: tail_final_mms(0, st0, 1)],
                    6: [lambda: tail_final_out(0, st0)],
                    18: [lambda: run_gen(tail_norm_q(1, st1, 0))],
                }, carry_in=carry)
                # terminal: interleave the normalize halves with the final
                # matmuls whose output chunks each half covers
                gen = tail_norm_q(1, st1, 1)
                next(gen)                        # cols 0:256 (chunks 0, 1)
                tail_final_mms(1, st1, 0, (0, 1))
                tail_final_mms(1, st1, 1, (0, 1))
                run_gen(gen)                     # cols 256:501 (chunks 2, 3)
                tail_final_mms(1, st1, 0, (2, 3))
                tail_final_mms(1, st1, 1, (2, 3))
                tail_final_out(1, st1)

    nc.compile()
    return nc


def _prep_weights(W_query, W_key, W_val, W1, W2, W3, W4, W_out):
    nf = 0.25  # 1/sqrt(16), exact power of two
    stack = lambda w: np.ascontiguousarray(
        np.asarray(w, np.float32).transpose(1, 0, 2).reshape(D, D))
    wq = stack(W_query) * nf
    wk = stack(W_key)
    mask = np.zeros((1, D), np.float32)
    for h in range(H):
        if h % 2 == 0:
            mask[0, h * KD:(h + 1) * KD] = 1.0
    wke = wk * mask
    wko = wk * (1.0 - mask)
    wo = np.asarray(W_out, np.float32)
    # W_out rows interleaved into 32-row groups: rows 32j+v hold head
    # (quad*4+j) vector v, rows 32j+16.. (denominator rows) are zero
    wo_pad = np.zeros((2, D, D), np.float32)
    for quad in range(2):
        for j in range(4):
            wo_pad[quad, 32 * j:32 * j + KD] = wo[quad * 4 + j]
    # sel[p, p'] = 1 iff p is the denominator row of p's 32-group
    sel = np.zeros((D, D), np.float32)
    for p2 in range(D):
        sel[32 * (p2 // 32) + 16, p2] = 1.0
    wall = np.concatenate(
        [wq, wke, wko, stack(W1) * nf, stack(W2) * nf, stack(W3) * nf,
         stack(W4) * nf, stack(W_val)], axis=1)
    wobf = np.concatenate([wo_pad[0], wo_pad[1]],
                          axis=1).astype(ml_dtypes.bfloat16)
    return {"wall": np.ascontiguousarray(wall.astype(ml_dtypes.bfloat16)),
            "wobf": np.ascontiguousarray(wobf),
            "sel": np.ascontiguousarray(sel)}


def prep_in_maps(inputs):
    """Full harness inputs -> per-core in_maps for run_bass_kernel_spmd."""
    w = _prep_weights(inputs["W_query"], inputs["W_key"], inputs["W_val"],
                      inputs["W1_query"], inputs["W2_query"],
                      inputs["W3_query"], inputs["W4_query"],
                      inputs["W_out"])
    q = np.asarray(inputs["q"], np.float32)
    hTr = np.ascontiguousarray(q[:, ROT, :].transpose(0, 2, 1))
    h_hi = hTr.astype(ml_dtypes.bfloat16)
    h_lo = (hTr - h_hi.astype(np.float32)).astype(ml_dtypes.bfloat16)
    hsplit = np.ascontiguousarray(np.stack([h_hi, h_lo], axis=1))
    return [dict(w, hq=hsplit[BPC * c:BPC * (c + 1)]) for c in range(NCORES)]


def _numpy_fallback(q, W_query, W_key, W_val, W1, W2, W3, W4, W_out,
                    n_pick, n_delivery):
    """Pure-numpy reference for unexpected n_pick/n_delivery (not used for
    the standard 250/250 problem)."""
    h = np.asarray(q, np.float64)
    Bq, Gq, _ = h.shape
    nf = 1.0 / math.sqrt(KD)
    NEG = -np.inf
    proj = lambda x, W: np.einsum("bnd,hdk->hbnk", x, np.asarray(W, np.float64))
    sc = lambda Q, K: nf * np.einsum("hbqk,hbgk->hbqg", Q, K)
    zm = lambda c: np.where(c == 0, NEG, c)
    Q, K, V = proj(h, W_query), proj(h, W_key), proj(h, W_val)
    comp = sc(Q, K)
    hp, hd = h[:, 1:1 + n_pick], h[:, 1 + n_pick:]
    Kp, Vp = proj(hp, W_key), proj(hp, W_val)
    Kd, Vd = proj(hd, W_key), proj(hd, W_val)
    c_pp = zm(sc(proj(hp, W1), Kp))
    c_pd = zm(sc(proj(hp, W2), Kd))
    c_dp = zm(sc(proj(hd, W3), Kp))
    c_dd = zm(sc(proj(hd, W4), Kd))

    def place(blk, r0):
        full = np.full((H, Bq, Gq, blk.shape[3]), NEG)
        full[:, :, r0:r0 + blk.shape[2], :] = blk
        return full

    md = hd.shape[1]
    cf = np.concatenate([comp, place(c_pp, 1), place(c_pd, 1),
                         place(c_dd, Gq - md), place(c_dp, Gq - md)], axis=-1)
    cf -= cf.max(axis=-1, keepdims=True)
    e = np.exp(cf)
    attn = e / e.sum(axis=-1, keepdims=True)
    g, mp = Gq, n_pick
    heads = np.einsum("hbqg,hbgv->hbqv", attn[..., :g], V)
    heads += np.einsum("hbqp,hbpv->hbqv", attn[..., g:g + mp], Vp)
    heads += np.einsum("hbqd,hbdv->hbqv", attn[..., g + mp:g + mp + md], Vd)
    heads += np.einsum("hbqd,hbdv->hbqv",
                       attn[..., g + mp + md:g + mp + 2 * md], Vd)
    heads += np.einsum("hbqp,hbpv->hbqv", attn[..., g + mp + 2 * md:], Vp)
    return np.einsum("hbqv,hve->bqe", heads,
                     np.asarray(W_out, np.float64)).astype(np.float32)


def kernel(q, W_query, W_key, W_val, W1_query, W2_query, W3_query, W4_query,
           W_out, n_pick, n_delivery):
    np_, nd_ = int(n_pick), int(n_delivery)
    q = np.asarray(q, np.float32)
    if np_ != NP or nd_ != ND or q.shape != (B, G, D):
        return _numpy_fallback(q, W_query, W_key, W_val, W1_query, W2_query,
                               W3_query, W4_query, W_out, np_, nd_)

    from concourse import bass_utils

    if "nc" not in _CACHE:
        _CACHE["nc"] = _build_nc()
    nc = _CACHE["nc"]

    w = _prep_weights(W_query, W_key, W_val, W1_query, W2_query, W3_query,
                      W4_query, W_out)
    # host layout: rotate g axis (picks, delivs, depot), transpose to
    # [b, d, g], and split into bf16 hi + lo residual so the device
    # projections run as two accumulating bf16 matmuls
    hTr = np.ascontiguousarray(q[:, ROT, :].transpose(0, 2, 1))
    h_hi = hTr.astype(ml_dtypes.bfloat16)
    h_lo = (hTr - h_hi.astype(np.float32)).astype(ml_dtypes.bfloat16)
    hsplit = np.ascontiguousarray(np.stack([h_hi, h_lo], axis=1))

    in_maps = [dict(w, hq=hsplit[BPC * c:BPC * (c + 1)])
               for c in range(NCORES)]
    res = bass_utils.run_bass_kernel_spmd(nc, in_maps,
                                          core_ids=list(range(NCORES)))
    return np.concatenate([r["out"] for r in res.results], axis=0)

